# revision 27
# baseline (speedup 1.0000x reference)
"""Trainium2 Bass kernel for attention + GroupNorm (nn_Attention_18992345383535).

Sharding: 8 cores = 4 batches x 2 sequence halves. Each core:
  - projects K, V for its batch over the full sequence (w_qkv columns 512:1536)
  - projects Q for its half of the sequence (scale folded into weights)
  - computes attention transposed: sim^T[j,i] = sum_d k[d,j] q[d,i], so the
    exp'd scores chain directly into the V matmul with no transposes
  - V is produced directly transposed (x as stationary operand), with a ones
    column appended so softmax row-sums fall out of the same matmul
  - output projection + bias, then per-channel [sum, sumsq] partial stats
A second tiny launch applies the GroupNorm affine (y*a + c) after the host
combines the per-channel partial sums into per-(batch, group) mean/var.

Matmul operands are float16 (1 cycle/row on the PE, 10-bit mantissa - the
same accuracy class as TF32 here since every value is well inside fp16
range); accumulation stays fp32 in PSUM. The exp() runs on the Scalar engine
reading sim straight from PSUM, which is the ~1 elem/lane/cycle floor this
kernel is scheduled around.
"""

import sys

sys.path.insert(0, "/opt/trn_rl_repo")

from contextlib import ExitStack

import numpy as np

import concourse.bass as bass
import concourse.bacc as bacc
import concourse.mybir as mybir
import concourse.tile as tile
from concourse.bass_utils import run_bass_kernel_spmd

F32 = mybir.dt.float32
F32R = mybir.dt.float32r
F16 = mybir.dt.float16
AX = mybir.AxisListType
OP = mybir.AluOpType
AF = mybir.ActivationFunctionType

B, C, N = 4, 512, 2048
HEADS, DH, HID = 8, 64, 512
NLOC = N // 2
GROUPS = 8
EPS = 1e-5
SCALE = DH**-0.5

TRACE = False
LAST_EXEC_NS = []


def build_main():
    nc = bacc.Bacc("TRN2", target_bir_lowering=False, debug=False, num_devices=8)
    x = nc.dram_tensor("x", [C, N], F16, kind="ExternalInput").ap()
    xq = nc.dram_tensor("xq", [C, NLOC], F16, kind="ExternalInput").ap()
    wqkvT = nc.dram_tensor("wqkvT", [C, 3 * HID], F16, kind="ExternalInput").ap()
    woutT = nc.dram_tensor("woutT", [HID, C], F16, kind="ExternalInput").ap()
    bout = nc.dram_tensor("bout", [C], F32, kind="ExternalInput").ap()
    y = nc.dram_tensor("y", [C, NLOC], F16, kind="ExternalOutput").ap()
    stats = nc.dram_tensor("stats", [C, 2], F32, kind="ExternalOutput").ap()

    yr = y.rearrange("(q p) i -> p q i", p=128)  # [128, 4, 1024]
    statsr = stats.rearrange("(q p) s -> p q s", p=128)  # [128, 4, 2]

    with tile.TileContext(nc) as tc, ExitStack() as ctx:
        const = ctx.enter_context(tc.tile_pool(name="const", bufs=1))
        work = ctx.enter_context(tc.tile_pool(name="work", bufs=2))
        psum = ctx.enter_context(tc.tile_pool(name="psum", bufs=3, space="PSUM"))

        wqr = wqkvT.rearrange("(c p) o -> p c o", p=128)
        xqrr = xq.rearrange("(c p) n -> p c n", p=128)
        xrr = x.rearrange("(c p) n -> p c n", p=128)
        wq_sb = const.tile([128, 4, 3 * HID], F16, tag="wqkv")
        xq_sb = const.tile([128, 4, NLOC], F16, tag="xq")
        x_sb = const.tile([128, 4, N], F16, tag="x")
        nc.sync.dma_start(out=wq_sb, in_=wqr)
        nc.sync.dma_start(out=xq_sb, in_=xqrr)
        nc.sync.dma_start(out=x_sb, in_=xrr)
        wo_sb = const.tile([128, 4, C], F16, tag="wout")
        nc.sync.dma_start(out=wo_sb, in_=woutT.rearrange("(h p) o -> p h o", p=128))
        bo_sb = const.tile([128, 4], F32, tag="bout")
        nc.sync.dma_start(out=bo_sb, in_=bout.rearrange("(q p) -> p q", p=128))

        K_sb = const.tile([128, 4, N], F16, tag="K")  # K[o, j], o = pair*128+p
        Q_sb = const.tile([128, 4, NLOC], F16, tag="Q")  # Q[o, i]
        # V^T per head, with an appended ones column (softmax row sums) and a
        # zero pad column.
        VT_sb = const.tile([128, 16, 8, 66], F16, tag="VT")
        AO_sb = const.tile([128, 4, NLOC], F16, tag="AO")  # attn out, hidden-major
        AOraw = const.tile([65, 8, NLOC], F32, tag="AOraw")
        Ysb = const.tile([128, 4, NLOC], F16, tag="Ysb")
        vtpad_f32 = const.tile([128, 16, 8, 2], F32, tag="vtpad")
        nc.vector.memset(vtpad_f32[:, :, :, 0:1], 1.0)
        nc.vector.memset(vtpad_f32[:, :, :, 1:2], 0.0)
        nc.vector.tensor_copy(out=VT_sb[:, :, :, 64:66], in_=vtpad_f32)

        rscr = nc.dram_tensor("rscr", [2, 4, 1024], F32).ap()
        rscr2 = nc.dram_tensor("rscr2", [2, 4, 1024], F32).ap()

        def emit_q_half(pair, half):
            ps = psum.tile([128, 512], F32, tag="mm", name=f"qp{pair}{half}")
            for c in range(4):
                nc.tensor.matmul(
                    ps,
                    lhsT=wq_sb[:, c, pair * 128 : (pair + 1) * 128],
                    rhs=xq_sb[:, c, half * 512 : (half + 1) * 512],
                    start=(c == 0),
                    stop=(c == 3),
                )
            nc.vector.tensor_copy(
                out=Q_sb[:, pair, half * 512 : (half + 1) * 512], in_=ps
            )

        def emit_k_chunk(pair, jc):
            ps = psum.tile([128, 512], F32, tag="mm", name=f"kp{pair}{jc}")
            for c in range(4):
                nc.tensor.matmul(
                    ps,
                    lhsT=wq_sb[:, c, HID + pair * 128 : HID + (pair + 1) * 128],
                    rhs=x_sb[:, c, jc * 512 : (jc + 1) * 512],
                    start=(c == 0),
                    stop=(c == 3),
                )
            nc.vector.tensor_copy(
                out=K_sb[:, pair, jc * 512 : (jc + 1) * 512], in_=ps
            )

        def emit_vt_block(jt):
            # two 128-col t-tiles of V^T
            ps = psum.tile([128, 1024], F32, tag="mm", name=f"vt{jt}")
            for half in range(2):
                for c in range(4):
                    nc.tensor.matmul(
                        ps[:, half * 512 : (half + 1) * 512],
                        lhsT=x_sb[:, c, jt * 256 + half * 128 : jt * 256 + (half + 1) * 128],
                        rhs=wq_sb[:, c, 2 * HID : 3 * HID],
                        start=(c == 0),
                        stop=(c == 3),
                    )
            nc.vector.tensor_copy(
                out=VT_sb[:, 2 * jt : 2 * jt + 2, :, 0:64],
                in_=ps.rearrange("p (t h c) -> p t h c", t=2, h=8),
            )

        st_parts = {}

        def attention(it, pair):
            isl = slice(it * 512, (it + 1) * 512)
            attnA = psum.tile([66, 512], F32, tag="attn", bufs=2, name=f"aA{it}{pair}")
            attnB = psum.tile([66, 512], F32, tag="attn", bufs=2, name=f"aB{it}{pair}")
            for j in range(16):
                sim = psum.tile([128, 1024], F32, tag="mm", name=f"s{it}{pair}{j}")
                nc.tensor.matmul(
                    sim[:, 0:512],
                    lhsT=K_sb[0:64, pair, j * 128 : (j + 1) * 128],
                    rhs=Q_sb[0:64, pair, isl],
                    start=True,
                    stop=True,
                    tile_position=(0, 0),
                )
                nc.tensor.matmul(
                    sim[:, 512:1024],
                    lhsT=K_sb[64:128, pair, j * 128 : (j + 1) * 128],
                    rhs=Q_sb[64:128, pair, isl],
                    start=True,
                    stop=True,
                    tile_position=(64, 0),
                )
                P = work.tile([128, 1024], F16, tag="P", bufs=6, name=f"P{it}{pair}{j}")
                nc.scalar.activation(out=P, in_=sim, func=AF.Exp)
                nc.tensor.matmul(
                    attnA,
                    lhsT=VT_sb[:, j, 2 * pair, :],
                    rhs=P[:, 0:512],
                    start=(j == 0),
                    stop=(j == 15),
                )
                nc.tensor.matmul(
                    attnB,
                    lhsT=VT_sb[:, j, 2 * pair + 1, :],
                    rhs=P[:, 512:1024],
                    start=(j == 0),
                    stop=(j == 15),
                )
            # softmax denominators straight from PSUM rows (parallel with the
            # AOraw copies), reciprocated in a [128, 8] layout via DRAM bounce
            nc.vector.tensor_copy(out=AOraw[:, 2 * pair, isl], in_=attnA[0:65, :])
            nc.sync.dma_start(out=rscr[it, pair, 0:512], in_=AOraw[64:65, 2 * pair, isl])
            nc.vector.tensor_copy(out=AOraw[:, 2 * pair + 1, isl], in_=attnB[0:65, :])
            nc.sync.dma_start(
                out=rscr[it, pair, 512:1024], in_=AOraw[64:65, 2 * pair + 1, isl]
            )
            Rt = work.tile([128, 8], F32, tag="Rt", name=f"Rt{it}{pair}")
            nc.sync.dma_start(
                out=Rt, in_=rscr[it, pair].rearrange("(p c) -> p c", p=128)
            )
            RtI = work.tile([128, 8], F32, tag="RtI", name=f"RtI{it}{pair}")
            nc.vector.reciprocal(out=RtI, in_=Rt)
            nc.sync.dma_start(
                out=rscr2[it, pair].rearrange("(p c) -> p c", p=128), in_=RtI
            )
            base = rscr2[it, pair]
            for hh in range(2):
                h = 2 * pair + hh
                Rbc = work.tile([64, 512], F32, tag="Rbc", bufs=3, name=f"Rb{it}{h}")
                bc_ap = bass.AP(
                    tensor=base.tensor,
                    offset=base.offset + hh * 512,
                    ap=[[0, 64], [1, 512]],
                )
                nc.sync.dma_start(out=Rbc, in_=bc_ap)
                if hh == 0:
                    nc.vector.tensor_mul(
                        out=AO_sb[0:64, pair, isl], in0=AOraw[0:64, h, isl], in1=Rbc
                    )
                else:
                    tmp = work.tile([64, 512], F16, tag="tmpb", bufs=2, name=f"t{it}{h}")
                    nc.vector.tensor_mul(out=tmp, in0=AOraw[0:64, h, isl], in1=Rbc)
                    nc.sync.dma_start(out=AO_sb[64:128, pair, isl], in_=tmp)

        def proj(it):
            isl = slice(it * 512, (it + 1) * 512)
            for q in range(4):
                ps = psum.tile([128, 512], F32, tag="mm", name=f"pr{it}{q}")
                for hp in range(4):
                    nc.tensor.matmul(
                        ps,
                        lhsT=wo_sb[:, hp, q * 128 : (q + 1) * 128],
                        rhs=AO_sb[:, hp, isl],
                        start=(hp == 0),
                        stop=(hp == 3),
                    )
                nc.vector.tensor_scalar_add(
                    out=Ysb[:, q, isl], in0=ps, scalar1=bo_sb[:, q : q + 1]
                )
                nc.sync.dma_start(out=yr[:, q, isl], in_=Ysb[:, q, isl])

        # ---- emission schedule: minimal critical prefix (Q/K slivers for
        # pair 0), everything else demoted so the static scheduler treats it
        # as PE gap filler behind the ACT-bound attention stream.
        emit_q_half(0, 0)
        emit_k_chunk(0, 0)
        with tc.high_priority(offset=-1000000):
            emit_k_chunk(0, 1)
            for jt in range(8):
                emit_vt_block(jt)
            emit_q_half(0, 1)
            emit_k_chunk(0, 2)
            emit_k_chunk(0, 3)
            for pair in range(1, 4):
                emit_q_half(pair, 0)
                emit_q_half(pair, 1)
                for jc in range(4):
                    emit_k_chunk(pair, jc)
        for pair in range(4):
            attention(0, pair)
        attention(1, 0)
        attention(1, 1)
        attention(1, 2)
        attention(1, 3)
        proj(0)
        proj(1)

        for q in range(4):
            st = work.tile([128, 2], F32, tag="st", name=f"st{q}")
            nc.vector.reduce_sum(out=st[:, 0:1], in_=Ysb[:, q, :], axis=AX.X)
            sq = work.tile([128, 1024], F32, tag="sq", name=f"sq{q}")
            nc.vector.tensor_mul(out=sq, in0=Ysb[:, q, :], in1=Ysb[:, q, :])
            nc.vector.reduce_sum(out=st[:, 1:2], in_=sq, axis=AX.X)
            nc.sync.dma_start(out=statsr[:, q, :], in_=st)

    nc.compile()
    return nc


def build_gn():
    nc = bacc.Bacc("TRN2", target_bir_lowering=False, debug=False, num_devices=8)
    yin = nc.dram_tensor("yin", [C, NLOC], F16, kind="ExternalInput").ap()
    a = nc.dram_tensor("a", [C], F32, kind="ExternalInput").ap()
    cc = nc.dram_tensor("c", [C], F32, kind="ExternalInput").ap()
    out = nc.dram_tensor("out", [C, NLOC], F32, kind="ExternalOutput").ap()
    yinr = yin.rearrange("(g p) i -> p g i", p=128)  # [128, 4, 1024]
    outr = out.rearrange("(g p) i -> p g i", p=128)
    with tile.TileContext(nc) as tc, ExitStack() as ctx:
        const = ctx.enter_context(tc.tile_pool(name="const", bufs=1))
        work = ctx.enter_context(tc.tile_pool(name="work", bufs=2))
        a_sb = const.tile([128, 4], F32, tag="a")
        nc.sync.dma_start(out=a_sb, in_=a.rearrange("(g p) -> p g", p=128))
        c_sb = const.tile([128, 4], F32, tag="c")
        nc.sync.dma_start(out=c_sb, in_=cc.rearrange("(g p) -> p g", p=128))
        for g2 in range(2):
            t = work.tile([128, 2, NLOC], F16, tag="t")
            nc.sync.dma_start(out=t, in_=yinr[:, 2 * g2 : 2 * g2 + 2, :])
            o = work.tile([128, 2, NLOC], F32, tag="o")
            for gg in range(2):
                g = 2 * g2 + gg
                nc.vector.tensor_scalar(
                    out=o[:, gg, :],
                    in0=t[:, gg, :],
                    scalar1=a_sb[:, g : g + 1],
                    scalar2=c_sb[:, g : g + 1],
                    op0=OP.mult,
                    op1=OP.add,
                )
            nc.sync.dma_start(out=outr[:, 2 * g2 : 2 * g2 + 2, :], in_=o)
    nc.compile()
    return nc


_CACHE = {}


def _get_programs():
    if "main" not in _CACHE:
        _CACHE["main"] = build_main()
        _CACHE["gn"] = build_gn()
    return _CACHE["main"], _CACHE["gn"]


def kernel(x, w_qkv, w_out, b_out, gn_w, gn_b):
    x = np.asarray(x, dtype=np.float32)
    w_qkv = np.asarray(w_qkv, dtype=np.float32)
    w_out = np.asarray(w_out, dtype=np.float32)
    b_out = np.ascontiguousarray(np.asarray(b_out, dtype=np.float32))
    gn_w = np.asarray(gn_w, dtype=np.float32)
    gn_b = np.asarray(gn_b, dtype=np.float32)

    ncm, ncg = _get_programs()

    wq = w_qkv.copy()
    wq[:HID] *= np.float32(SCALE)
    wqkvT = np.ascontiguousarray(wq.T.astype(np.float16))
    woutT = np.ascontiguousarray(w_out.T.astype(np.float16))

    in_maps = []
    for b in range(B):
        xb = np.ascontiguousarray(x[b].astype(np.float16))
        for s in range(2):
            in_maps.append(
                {
                    "x": xb,
                    "xq": np.ascontiguousarray(xb[:, s * NLOC : (s + 1) * NLOC]),
                    "wqkvT": wqkvT,
                    "woutT": woutT,
                    "bout": b_out,
                }
            )
    r1 = run_bass_kernel_spmd(ncm, in_maps, core_ids=list(range(8)), trace=TRACE)
    if TRACE:
        LAST_EXEC_NS.append(r1.exec_time_ns)

    in2 = []
    for b in range(B):
        st = r1.results[2 * b]["stats"].astype(np.float64) + r1.results[2 * b + 1][
            "stats"
        ].astype(np.float64)
        g = st.reshape(GROUPS, C // GROUPS, 2).sum(axis=1)  # [8, 2]
        ntot = (C // GROUPS) * N
        mean = g[:, 0] / ntot
        var = g[:, 1] / ntot - mean**2
        rstd = 1.0 / np.sqrt(var + EPS)
        a = gn_w.astype(np.float64) * np.repeat(rstd, C // GROUPS)
        c = gn_b.astype(np.float64) - np.repeat(mean, C // GROUPS) * a
        a32 = np.ascontiguousarray(a.astype(np.float32))
        c32 = np.ascontiguousarray(c.astype(np.float32))
        for s in range(2):
            in2.append({"yin": r1.results[2 * b + s]["y"], "a": a32, "c": c32})
    r2 = run_bass_kernel_spmd(ncg, in2, core_ids=list(range(8)), trace=TRACE)
    if TRACE:
        LAST_EXEC_NS.append(r2.exec_time_ns)

    out = np.empty((B, C, N), dtype=np.float32)
    for b in range(B):
        for s in range(2):
            out[b, :, s * NLOC : (s + 1) * NLOC] = r2.results[2 * b + s]["out"]
    return out


# revision 28
# speedup vs baseline: 1.0096x; 1.0096x over previous
"""Trainium2 Bass kernel for attention + GroupNorm (nn_Attention_18992345383535).

Sharding: 8 cores = 4 batches x 2 sequence halves. Each core:
  - projects K, V for its batch over the full sequence (w_qkv columns 512:1536)
  - projects Q for its half of the sequence (scale folded into weights)
  - computes attention transposed: sim^T[j,i] = sum_d k[d,j] q[d,i], so the
    exp'd scores chain directly into the V matmul with no transposes
  - V is produced directly transposed (x as stationary operand), with a ones
    column appended so softmax row-sums fall out of the same matmul
  - output projection + bias, then per-channel [sum, sumsq] partial stats
A second tiny launch applies the GroupNorm affine (y*a + c) after the host
combines the per-channel partial sums into per-(batch, group) mean/var.

Matmul operands are float16 (1 cycle/row on the PE, 10-bit mantissa - the
same accuracy class as TF32 here since every value is well inside fp16
range); accumulation stays fp32 in PSUM. The exp() runs on the Scalar engine
reading sim straight from PSUM, which is the ~1 elem/lane/cycle floor this
kernel is scheduled around.
"""

import sys

sys.path.insert(0, "/opt/trn_rl_repo")

from contextlib import ExitStack

import numpy as np

import concourse.bass as bass
import concourse.bacc as bacc
import concourse.mybir as mybir
import concourse.tile as tile
from concourse.bass_utils import run_bass_kernel_spmd

F32 = mybir.dt.float32
F32R = mybir.dt.float32r
F16 = mybir.dt.float16
AX = mybir.AxisListType
OP = mybir.AluOpType
AF = mybir.ActivationFunctionType

B, C, N = 4, 512, 2048
HEADS, DH, HID = 8, 64, 512
NLOC = N // 2
GROUPS = 8
EPS = 1e-5
SCALE = DH**-0.5

TRACE = False
LAST_EXEC_NS = []


def build_main():
    nc = bacc.Bacc("TRN2", target_bir_lowering=False, debug=False, num_devices=8)
    x = nc.dram_tensor("x", [C, N], F16, kind="ExternalInput").ap()
    wqkvT = nc.dram_tensor("wqkvT", [C, 3 * HID], F16, kind="ExternalInput").ap()
    woutT = nc.dram_tensor("woutT", [HID, C], F16, kind="ExternalInput").ap()
    bout = nc.dram_tensor("bout", [C], F32, kind="ExternalInput").ap()
    y = nc.dram_tensor("y", [C, NLOC], F16, kind="ExternalOutput").ap()
    stats = nc.dram_tensor("stats", [C, 2], F32, kind="ExternalOutput").ap()

    yr = y.rearrange("(q p) i -> p q i", p=128)  # [128, 4, 1024]
    statsr = stats.rearrange("(q p) s -> p q s", p=128)  # [128, 4, 2]

    with tile.TileContext(nc) as tc, ExitStack() as ctx:
        const = ctx.enter_context(tc.tile_pool(name="const", bufs=1))
        work = ctx.enter_context(tc.tile_pool(name="work", bufs=2))
        psum = ctx.enter_context(tc.tile_pool(name="psum", bufs=3, space="PSUM"))

        wqr = wqkvT.rearrange("(c p) o -> p c o", p=128)
        xrr = x.rearrange("(c p) n -> p c n", p=128)
        wq_sb = const.tile([128, 4, 3 * HID], F16, tag="wqkv")
        x_sb = const.tile([128, 4, N], F16, tag="x")
        nc.sync.dma_start(out=wq_sb, in_=wqr)
        nc.sync.dma_start(out=x_sb, in_=xrr)
        wo_sb = const.tile([128, 4, C], F16, tag="wout")
        nc.sync.dma_start(out=wo_sb, in_=woutT.rearrange("(h p) o -> p h o", p=128))
        bo_sb = const.tile([128, 4], F32, tag="bout")
        nc.sync.dma_start(out=bo_sb, in_=bout.rearrange("(q p) -> p q", p=128))

        K_sb = const.tile([128, 4, N], F16, tag="K")  # K[o, j], o = pair*128+p
        Q_sb = const.tile([128, 4, NLOC], F16, tag="Q")  # Q[o, i]
        # V^T per head, with an appended ones column (softmax row sums) and a
        # zero pad column.
        VT_sb = const.tile([128, 16, 8, 66], F16, tag="VT")
        AO_sb = const.tile([128, 4, NLOC], F16, tag="AO")  # attn out, hidden-major
        AOraw = const.tile([65, 8, NLOC], F32, tag="AOraw")
        Ysb = const.tile([128, 4, NLOC], F16, tag="Ysb")
        vtpad_f32 = const.tile([128, 16, 8, 2], F32, tag="vtpad")
        nc.vector.memset(vtpad_f32[:, :, :, 0:1], 1.0)
        nc.vector.memset(vtpad_f32[:, :, :, 1:2], 0.0)
        nc.vector.tensor_copy(out=VT_sb[:, :, :, 64:66], in_=vtpad_f32)

        rscr = nc.dram_tensor("rscr", [2, 4, 1024], F32).ap()
        rscr2 = nc.dram_tensor("rscr2", [2, 4, 1024], F32).ap()

        def emit_q_half(pair, half):
            ps = psum.tile([128, 512], F32, tag="mm", name=f"qp{pair}{half}")
            for c in range(4):
                nc.tensor.matmul(
                    ps,
                    lhsT=wq_sb[:, c, pair * 128 : (pair + 1) * 128],
                    rhs=x_sb[:, c, half * 512 : (half + 1) * 512],
                    start=(c == 0),
                    stop=(c == 3),
                )
            nc.vector.tensor_copy(
                out=Q_sb[:, pair, half * 512 : (half + 1) * 512], in_=ps
            )

        def emit_k_chunk(pair, jc):
            ps = psum.tile([128, 512], F32, tag="mm", name=f"kp{pair}{jc}")
            for c in range(4):
                nc.tensor.matmul(
                    ps,
                    lhsT=wq_sb[:, c, HID + pair * 128 : HID + (pair + 1) * 128],
                    rhs=x_sb[:, c, jc * 512 : (jc + 1) * 512],
                    start=(c == 0),
                    stop=(c == 3),
                )
            nc.vector.tensor_copy(
                out=K_sb[:, pair, jc * 512 : (jc + 1) * 512], in_=ps
            )

        def emit_vt_block(jt):
            # two 128-col t-tiles of V^T
            ps = psum.tile([128, 1024], F32, tag="mm", name=f"vt{jt}")
            for half in range(2):
                for c in range(4):
                    nc.tensor.matmul(
                        ps[:, half * 512 : (half + 1) * 512],
                        lhsT=x_sb[:, c, jt * 256 + half * 128 : jt * 256 + (half + 1) * 128],
                        rhs=wq_sb[:, c, 2 * HID : 3 * HID],
                        start=(c == 0),
                        stop=(c == 3),
                    )
            nc.vector.tensor_copy(
                out=VT_sb[:, 2 * jt : 2 * jt + 2, :, 0:64],
                in_=ps.rearrange("p (t h c) -> p t h c", t=2, h=8),
            )

        st_parts = {}

        def attention(it, pair):
            isl = slice(it * 512, (it + 1) * 512)
            attnA = psum.tile([66, 512], F32, tag="attn", bufs=2, name=f"aA{it}{pair}")
            attnB = psum.tile([66, 512], F32, tag="attn", bufs=2, name=f"aB{it}{pair}")
            for j in range(16):
                sim = psum.tile([128, 1024], F32, tag="mm", name=f"s{it}{pair}{j}")
                nc.tensor.matmul(
                    sim[:, 0:512],
                    lhsT=K_sb[0:64, pair, j * 128 : (j + 1) * 128],
                    rhs=Q_sb[0:64, pair, isl],
                    start=True,
                    stop=True,
                    tile_position=(0, 0),
                )
                nc.tensor.matmul(
                    sim[:, 512:1024],
                    lhsT=K_sb[64:128, pair, j * 128 : (j + 1) * 128],
                    rhs=Q_sb[64:128, pair, isl],
                    start=True,
                    stop=True,
                    tile_position=(64, 0),
                )
                P = work.tile([128, 1024], F16, tag="P", bufs=6, name=f"P{it}{pair}{j}")
                nc.scalar.activation(out=P, in_=sim, func=AF.Exp)
                nc.tensor.matmul(
                    attnA,
                    lhsT=VT_sb[:, j, 2 * pair, :],
                    rhs=P[:, 0:512],
                    start=(j == 0),
                    stop=(j == 15),
                )
                nc.tensor.matmul(
                    attnB,
                    lhsT=VT_sb[:, j, 2 * pair + 1, :],
                    rhs=P[:, 512:1024],
                    start=(j == 0),
                    stop=(j == 15),
                )
            # softmax denominators straight from PSUM rows (parallel with the
            # AOraw copies), reciprocated in a [128, 8] layout via DRAM bounce
            nc.vector.tensor_copy(out=AOraw[:, 2 * pair, isl], in_=attnA[0:65, :])
            nc.sync.dma_start(out=rscr[it, pair, 0:512], in_=AOraw[64:65, 2 * pair, isl])
            nc.vector.tensor_copy(out=AOraw[:, 2 * pair + 1, isl], in_=attnB[0:65, :])
            nc.sync.dma_start(
                out=rscr[it, pair, 512:1024], in_=AOraw[64:65, 2 * pair + 1, isl]
            )
            Rt = work.tile([128, 8], F32, tag="Rt", name=f"Rt{it}{pair}")
            nc.sync.dma_start(
                out=Rt, in_=rscr[it, pair].rearrange("(p c) -> p c", p=128)
            )
            RtI = work.tile([128, 8], F32, tag="RtI", name=f"RtI{it}{pair}")
            nc.vector.reciprocal(out=RtI, in_=Rt)
            nc.sync.dma_start(
                out=rscr2[it, pair].rearrange("(p c) -> p c", p=128), in_=RtI
            )
            base = rscr2[it, pair]
            for hh in range(2):
                h = 2 * pair + hh
                Rbc = work.tile([64, 512], F32, tag="Rbc", bufs=3, name=f"Rb{it}{h}")
                bc_ap = bass.AP(
                    tensor=base.tensor,
                    offset=base.offset + hh * 512,
                    ap=[[0, 64], [1, 512]],
                )
                nc.sync.dma_start(out=Rbc, in_=bc_ap)
                if hh == 0:
                    nc.vector.tensor_mul(
                        out=AO_sb[0:64, pair, isl], in0=AOraw[0:64, h, isl], in1=Rbc
                    )
                else:
                    tmp = work.tile([64, 512], F16, tag="tmpb", bufs=2, name=f"t{it}{h}")
                    nc.vector.tensor_mul(out=tmp, in0=AOraw[0:64, h, isl], in1=Rbc)
                    nc.sync.dma_start(out=AO_sb[64:128, pair, isl], in_=tmp)

        def proj(it):
            isl = slice(it * 512, (it + 1) * 512)
            for q in range(4):
                ps = psum.tile([128, 512], F32, tag="mm", name=f"pr{it}{q}")
                for hp in range(4):
                    nc.tensor.matmul(
                        ps,
                        lhsT=wo_sb[:, hp, q * 128 : (q + 1) * 128],
                        rhs=AO_sb[:, hp, isl],
                        start=(hp == 0),
                        stop=(hp == 3),
                    )
                nc.vector.tensor_scalar_add(
                    out=Ysb[:, q, isl], in0=ps, scalar1=bo_sb[:, q : q + 1]
                )
                nc.sync.dma_start(out=yr[:, q, isl], in_=Ysb[:, q, isl])

        # ---- emission schedule: minimal critical prefix (Q/K slivers for
        # pair 0), everything else demoted so the static scheduler treats it
        # as PE gap filler behind the ACT-bound attention stream.
        emit_q_half(0, 0)
        emit_k_chunk(0, 0)
        with tc.high_priority(offset=-1000000):
            emit_k_chunk(0, 1)
            for jt in range(8):
                emit_vt_block(jt)
            emit_q_half(0, 1)
            emit_k_chunk(0, 2)
            emit_k_chunk(0, 3)
            for pair in range(1, 4):
                emit_q_half(pair, 0)
                emit_q_half(pair, 1)
                for jc in range(4):
                    emit_k_chunk(pair, jc)
        for pair in range(4):
            attention(0, pair)
        attention(1, 0)
        attention(1, 1)
        attention(1, 2)
        attention(1, 3)
        proj(0)
        proj(1)

        for q in range(4):
            st = work.tile([128, 2], F32, tag="st", name=f"st{q}")
            nc.vector.reduce_sum(out=st[:, 0:1], in_=Ysb[:, q, :], axis=AX.X)
            sq = work.tile([128, 1024], F32, tag="sq", name=f"sq{q}")
            nc.vector.tensor_mul(out=sq, in0=Ysb[:, q, :], in1=Ysb[:, q, :])
            nc.vector.reduce_sum(out=st[:, 1:2], in_=sq, axis=AX.X)
            nc.sync.dma_start(out=statsr[:, q, :], in_=st)

    nc.compile()
    return nc


def build_gn():
    nc = bacc.Bacc("TRN2", target_bir_lowering=False, debug=False, num_devices=8)
    yin = nc.dram_tensor("yin", [C, NLOC], F16, kind="ExternalInput").ap()
    a = nc.dram_tensor("a", [C], F32, kind="ExternalInput").ap()
    cc = nc.dram_tensor("c", [C], F32, kind="ExternalInput").ap()
    out = nc.dram_tensor("out", [C, NLOC], F32, kind="ExternalOutput").ap()
    yinr = yin.rearrange("(g p) i -> p g i", p=128)  # [128, 4, 1024]
    outr = out.rearrange("(g p) i -> p g i", p=128)
    with tile.TileContext(nc) as tc, ExitStack() as ctx:
        const = ctx.enter_context(tc.tile_pool(name="const", bufs=1))
        work = ctx.enter_context(tc.tile_pool(name="work", bufs=2))
        a_sb = const.tile([128, 4], F32, tag="a")
        nc.sync.dma_start(out=a_sb, in_=a.rearrange("(g p) -> p g", p=128))
        c_sb = const.tile([128, 4], F32, tag="c")
        nc.sync.dma_start(out=c_sb, in_=cc.rearrange("(g p) -> p g", p=128))
        for g2 in range(2):
            t = work.tile([128, 2, NLOC], F16, tag="t")
            nc.sync.dma_start(out=t, in_=yinr[:, 2 * g2 : 2 * g2 + 2, :])
            o = work.tile([128, 2, NLOC], F32, tag="o")
            for gg in range(2):
                g = 2 * g2 + gg
                nc.vector.tensor_scalar(
                    out=o[:, gg, :],
                    in0=t[:, gg, :],
                    scalar1=a_sb[:, g : g + 1],
                    scalar2=c_sb[:, g : g + 1],
                    op0=OP.mult,
                    op1=OP.add,
                )
            nc.sync.dma_start(out=outr[:, 2 * g2 : 2 * g2 + 2, :], in_=o)
    nc.compile()
    return nc


_CACHE = {}


def _get_programs():
    if "main" not in _CACHE:
        _CACHE["main"] = build_main()
        _CACHE["gn"] = build_gn()
    return _CACHE["main"], _CACHE["gn"]


def kernel(x, w_qkv, w_out, b_out, gn_w, gn_b):
    x = np.asarray(x, dtype=np.float32)
    w_qkv = np.asarray(w_qkv, dtype=np.float32)
    w_out = np.asarray(w_out, dtype=np.float32)
    b_out = np.ascontiguousarray(np.asarray(b_out, dtype=np.float32))
    gn_w = np.asarray(gn_w, dtype=np.float32)
    gn_b = np.asarray(gn_b, dtype=np.float32)

    ncm, ncg = _get_programs()

    wq = w_qkv.copy()
    wq[:HID] *= np.float32(SCALE)
    wqkvT = np.ascontiguousarray(wq.T.astype(np.float16))
    woutT = np.ascontiguousarray(w_out.T.astype(np.float16))

    in_maps = []
    for b in range(B):
        xb = x[b].astype(np.float16)
        for s in range(2):
            # query half first; key order is permutation-invariant
            xrot = np.ascontiguousarray(
                np.concatenate(
                    [xb[:, s * NLOC : (s + 1) * NLOC], xb[:, (1 - s) * NLOC : (2 - s) * NLOC]],
                    axis=1,
                )
            )
            in_maps.append(
                {
                    "x": xrot,
                    "wqkvT": wqkvT,
                    "woutT": woutT,
                    "bout": b_out,
                }
            )
    r1 = run_bass_kernel_spmd(ncm, in_maps, core_ids=list(range(8)), trace=TRACE)
    if TRACE:
        LAST_EXEC_NS.append(r1.exec_time_ns)

    in2 = []
    for b in range(B):
        st = r1.results[2 * b]["stats"].astype(np.float64) + r1.results[2 * b + 1][
            "stats"
        ].astype(np.float64)
        g = st.reshape(GROUPS, C // GROUPS, 2).sum(axis=1)  # [8, 2]
        ntot = (C // GROUPS) * N
        mean = g[:, 0] / ntot
        var = g[:, 1] / ntot - mean**2
        rstd = 1.0 / np.sqrt(var + EPS)
        a = gn_w.astype(np.float64) * np.repeat(rstd, C // GROUPS)
        c = gn_b.astype(np.float64) - np.repeat(mean, C // GROUPS) * a
        a32 = np.ascontiguousarray(a.astype(np.float32))
        c32 = np.ascontiguousarray(c.astype(np.float32))
        for s in range(2):
            in2.append({"yin": r1.results[2 * b + s]["y"], "a": a32, "c": c32})
    r2 = run_bass_kernel_spmd(ncg, in2, core_ids=list(range(8)), trace=TRACE)
    if TRACE:
        LAST_EXEC_NS.append(r2.exec_time_ns)

    out = np.empty((B, C, N), dtype=np.float32)
    for b in range(B):
        for s in range(2):
            out[b, :, s * NLOC : (s + 1) * NLOC] = r2.results[2 * b + s]["out"]
    return out
